# revision 4
# baseline (speedup 1.0000x reference)
"""Trainium2 Bass kernel for nn_MemoryLayer (scatter_memory).

Reference computation (per token, N = B*S = 8192 tokens):
  z = x @ W_proj + b_proj                  # [N, 640]
  factor = sigmoid(2*|z|)  (== (1+tanh|z|)/2), per element
  score[n, t] = prod_l factor[n, t*10+l]   # [N, 64]
  code[n, t]  = sum_l (z[n, t*10+l] > 0) * 2^l   # bucket in [0, 1024)
  out[n] = sum_t tables[t*1024 + code[n,t]] * score[n,t] + bias

Sharding: data-parallel over tokens (1024 tokens per core, 8 cores).

Design (v3):
- Tables stored in HBM as fp8 E3M4 (x128 scale): the gather stream stays
  at 1 byte/elem AND gathered rows feed PE / ACT / DVE directly, with no
  mandatory int8->fp16 upconversion pass.
- z projection runs as a 3-pass bf16 split (xh@Wh + xh@Wl + xl@Wh),
  4/3x faster than the PE's 4-pass true-fp32 mode at ~fp32 accuracy
  (sign(z) flips vs the f32 reference are what matter; measured end to
  end this adds <4e-4 to the output rel err).
- Token tiles are processed as 8 single-tile groups (128 tokens each),
  software-pipelined 2 groups deep: while group k's tables accumulate,
  group k+2's projection runs, so the gather DMA stream never starves.
- Gathers batch 8 tables per SWDGE call (1024 rows of 1KB): descgen is
  994ns fixed + 0.34ns/descriptor, so big calls amortize the fixed cost
  (64 calls/rep ~ 86us vs 150us for 128 calls).
- The weighted sum over the 64 tables is split across three statically
  balanced engine paths (per table t, per group):
    A: ACT builds diag(score) fp16; PE: psum += diag.T @ g_fp8
    B: ACT: gs = fp16(g * score) fused;  PE: psum += I.T @ gs
    C: DVE: sacc += g * (score * 2^-7)   (scalar_tensor_tensor, f32)
  Merge per group: ACT: o = psum * 2^-7; GPSIMD: o += sacc.
- score = prod_l sigmoid(2|z|) via DVE multiply-reduce (no Ln/Exp), so
  ACT only uses {Abs, Sigmoid, Copy} -- one activation table set, no
  LoadActFuncSet churn.

dma_gather consumes int16 indices in a 16-partition wrapped layout:
unwrapped[i] = idxs[i % 16, i // 16], row i lands at out[i % 128,
i // 128, :].  With position i = j*128 + p (j = table-in-block,
p = partition/token), the required index column is c = j*8 + p//16 with
value code[p, t=8b+j] + 1024*j.  The cross-partition shuffle
idx[p%16 (replicated), j*8 + p//16] = code[p] is done on the tensor
engine with 8 selector matmuls (SELR_q[p, m] = [p == q*16 + m%16]), and
a strided DVE scalar_tensor_tensor writes the int16 indices while adding
the 1024*j block offset.
"""

import numpy as np
import ml_dtypes

import concourse.bacc as bacc
import concourse.bass as bass
import concourse.mybir as mybir
import concourse.tile as tile
from concourse.bass_utils import run_bass_kernel_spmd

# Problem constants (hardcoded per contest rules).
B, S = 4, 2048
HIDDEN = 1024
OUT = 1024
NUM_TABLE = 64
CODE_LEN = 10
TABLE_SIZE = 1024
TOTAL_DIM = NUM_TABLE * CODE_LEN  # 640
HALF_DIM = TOTAL_DIM // 2         # 320 = 32 tables

N_CORES = 8
N_TOKENS = B * S              # 8192
TOK = N_TOKENS // N_CORES     # 1024 tokens per core
P = 128                       # partitions
NT = TOK // P                 # 8 token tiles (= pipeline groups) per core
KCH = HIDDEN // P             # 8 contraction chunks
NBLK = 8                      # tables per gather call
NCALL = NUM_TABLE // NBLK     # 8 gather calls per group

FP8_SCALE = 128.0             # tables stored as e3m4(v * 128)
DEQ = 1.0 / FP8_SCALE

# Static path assignment per table: A (PE+diag), B (ACT conv + PE id),
# C (DVE sbuf accumulate).  Balanced from the cost model: per group-tile
# ACT ~946ns (B) / 199ns (A diag), PE ~483ns (A, B), DVE ~1127ns (C).
N_A, N_B, N_C = 24, 20, 20


def _make_paths():
    # Weighted round-robin spread of A/B/C over the 64 tables.
    counts = {"A": N_A, "B": N_B, "C": N_C}
    used = {k: 0 for k in counts}
    paths = []
    for t in range(NUM_TABLE):
        best, bestv = None, None
        for kk, c in counts.items():
            v = c * (t + 1) / NUM_TABLE - used[kk]
            if bestv is None or v > bestv:
                best, bestv = kk, v
        paths.append(best)
        used[best] += 1
    return paths


PATHS = _make_paths()
PE_TABLES = [t for t in range(NUM_TABLE) if PATHS[t] in ("A", "B")]
C_TABLES = [t for t in range(NUM_TABLE) if PATHS[t] == "C"]
PE_FIRST, PE_LAST = PE_TABLES[0], PE_TABLES[-1]
C_FIRST = C_TABLES[0] if C_TABLES else -1

dt = mybir.dt
Alu = mybir.AluOpType
Act = mybir.ActivationFunctionType
Axis = mybir.AxisListType


def emit_device_kernel(tc, out_ap, ins, two_queues=True):
    """Emit the per-core kernel. ins is a dict name -> bass.AP."""
    nc = tc.nc
    xh = ins["xh"]          # [1024 hidden, 1024 tok] bf16 (pre-transposed)
    xl = ins["xl"]          # [1024, 1024] bf16 residual
    Wh = ins["Wh"]          # [1024, 640] bf16
    Wl = ins["Wl"]          # [1024, 640] bf16 residual
    bph = ins["bph"]        # [1, 640] bf16
    bpl = ins["bpl"]        # [1, 640] bf16 residual
    tabs = ins["tabs"]      # [65536, 1024] int8 bytes == e3m4(v*128)
    Pm = ins["pmat"]        # [128, 640] f32  (2^l pattern, replicated rows)
    id16 = ins["id16"]      # [128, 128] f16 identity
    selr = ins["selr"]      # [128, 8, 128] f16 selector mats for idx shuffle
    offp = ins["offpat"]    # [128, 64] f32: 1024*(col % 8) block offsets
    # out_ap: [128, 8, 1024] f32; token d = k*128 + partition

    from contextlib import ExitStack

    with ExitStack() as ctx:
        const = ctx.enter_context(tc.tile_pool(name="const", bufs=1))

        xh_sb = const.tile([P, KCH, TOK], dt.bfloat16)
        xl_sb = const.tile([P, KCH, TOK], dt.bfloat16)
        for k in range(NT):
            # split per token tile so the first z matmul starts early
            sl = slice(k * P, (k + 1) * P)
            nc.sync.dma_start(
                xh_sb[:, :, sl], xh[:, sl].rearrange("(c p) h -> p c h", c=KCH))
            nc.sync.dma_start(
                xl_sb[:, :, sl], xl[:, sl].rearrange("(c p) h -> p c h", c=KCH))
        Wh_sb = const.tile([P, KCH, TOTAL_DIM], dt.bfloat16)
        nc.sync.dma_start(Wh_sb[:], Wh[:].rearrange("(c p) h -> p c h", c=KCH))
        Wl_sb = const.tile([P, KCH, TOTAL_DIM], dt.bfloat16)
        nc.sync.dma_start(Wl_sb[:], Wl[:].rearrange("(c p) h -> p c h", c=KCH))
        Pm_sb = const.tile([P, TOTAL_DIM], dt.float32)
        nc.sync.dma_start(Pm_sb[:], Pm[:])
        id16_sb = const.tile([P, P], dt.float16)
        nc.sync.dma_start(id16_sb[:], id16[:])
        selr_sb = const.tile([P, NT, P], dt.float16)
        nc.sync.dma_start(selr_sb[:], selr[:])
        offp_sb = const.tile([P, NUM_TABLE], dt.float32)
        nc.sync.dma_start(offp_sb[:], offp[:])
        bph_sb = const.tile([1, TOTAL_DIM], dt.bfloat16)
        nc.sync.dma_start(bph_sb[:], bph[:])
        bpl_sb = const.tile([1, TOTAL_DIM], dt.bfloat16)
        nc.sync.dma_start(bpl_sb[:], bpl[:])
        ones_sb = const.tile([1, P], dt.bfloat16)
        nc.vector.memset(ones_sb[:], 1.0)

        score_sb = const.tile([P, NUM_TABLE, NT], dt.float32)
        scoreq_sb = const.tile([P, NUM_TABLE, NT], dt.float32)

        # SBUF working pools (all open together; groups pipeline through)
        gpool = ctx.enter_context(tc.tile_pool(name="gbuf", bufs=3))
        gspool = ctx.enter_context(tc.tile_pool(name="gsc", bufs=4))
        dgpool = ctx.enter_context(tc.tile_pool(name="diag", bufs=8))
        sapool = ctx.enter_context(tc.tile_pool(name="sacc", bufs=3))
        opool = ctx.enter_context(tc.tile_pool(name="outs", bufs=3))
        cdpool = ctx.enter_context(tc.tile_pool(name="code", bufs=3))
        ixpool = ctx.enter_context(tc.tile_pool(name="idx", bufs=3))
        ztmp = ctx.enter_context(tc.tile_pool(name="ztmp", bufs=2))
        # PSUM: zA(2) + zB(1) + shuf(1) + acc(2x2) = 8 banks
        zpsum = ctx.enter_context(tc.tile_pool(name="zp", bufs=1, space="PSUM"))
        accp = ctx.enter_context(tc.tile_pool(name="acc", bufs=2, space="PSUM"))

        zA = {}
        zB = {}
        idx16 = {}
        codes = {}

        def phase1_z(k):
            """z[:, :320] and z[:, 320:] for token tile k (bf16 3-pass)."""
            za = zpsum.tile([P, HALF_DIM], dt.float32, tag="zA", bufs=2)
            zb = zpsum.tile([P, HALF_DIM], dt.float32, tag="zB", bufs=1)
            zA[k], zB[k] = za, zb
            sl = slice(k * P, (k + 1) * P)
            for c in range(KCH):
                first = c == 0
                nc.tensor.matmul(za[:], xh_sb[:, c, sl], Wh_sb[:, c, :HALF_DIM],
                                 start=first, stop=False)
                nc.tensor.matmul(za[:], xh_sb[:, c, sl], Wl_sb[:, c, :HALF_DIM],
                                 start=False, stop=False)
                nc.tensor.matmul(zb[:], xh_sb[:, c, sl], Wh_sb[:, c, HALF_DIM:],
                                 start=first, stop=False)
                nc.tensor.matmul(zb[:], xh_sb[:, c, sl], Wl_sb[:, c, HALF_DIM:],
                                 start=False, stop=False)
                nc.tensor.matmul(za[:], xl_sb[:, c, sl], Wh_sb[:, c, :HALF_DIM],
                                 start=False, stop=False)
                nc.tensor.matmul(zb[:], xl_sb[:, c, sl], Wh_sb[:, c, HALF_DIM:],
                                 start=False, stop=False)
            # bias (hi+lo) via rank-1 ones trick
            nc.tensor.matmul(za[:], ones_sb[0:1, :], bph_sb[0:1, :HALF_DIM],
                             start=False, stop=False)
            nc.tensor.matmul(zb[:], ones_sb[0:1, :], bph_sb[0:1, HALF_DIM:],
                             start=False, stop=False)
            nc.tensor.matmul(za[:], ones_sb[0:1, :], bpl_sb[0:1, :HALF_DIM],
                             start=False, stop=True)
            nc.tensor.matmul(zb[:], ones_sb[0:1, :], bpl_sb[0:1, HALF_DIM:],
                             start=False, stop=True)

        def phase1_post(k):
            """score/scoreq/code + wrapped int16 gather indices for tile k."""
            code_k = cdpool.tile([P, NUM_TABLE], dt.float16, tag="code")
            codes[k] = code_k
            for h, z in ((0, zA[k]), (1, zB[k])):
                tsl = slice(32 * h, 32 * (h + 1))
                dsl = slice(HALF_DIM * h, HALF_DIM * (h + 1))
                ab = ztmp.tile([P, HALF_DIM], dt.float32, tag="ab")
                nc.scalar.activation(ab[:], z[:], Act.Abs)
                fa = ztmp.tile([P, HALF_DIM], dt.float32, tag="fa")
                nc.scalar.activation(fa[:], ab[:], Act.Sigmoid, scale=2.0)
                # score = prod_l sigmoid(2|z|): DVE multiply-reduce
                nc.vector.tensor_reduce(
                    score_sb[:, tsl, k],
                    fa[:].rearrange("p (t l) -> p t l", l=CODE_LEN),
                    axis=Axis.X, op=Alu.mult,
                )
                # scoreq = score * 2^-7 (fp8 dequant) for the DVE C path
                nc.vector.tensor_scalar(
                    scoreq_sb[:, tsl, k], score_sb[:, tsl, k], DEQ, None,
                    op0=Alu.mult)
                bc = ztmp.tile([P, HALF_DIM], dt.float32, tag="bc")
                nc.vector.scalar_tensor_tensor(
                    bc[:], z[:], 0.0, Pm_sb[:, dsl],
                    op0=Alu.is_gt, op1=Alu.mult)
                with nc.allow_low_precision(
                        reason="codes are integers <= 1023, exact in fp16"):
                    nc.vector.tensor_reduce(
                        code_k[:, tsl],
                        bc[:].rearrange("p (t l) -> p t l", l=CODE_LEN),
                        axis=Axis.X, op=Alu.add,
                    )
            # wrapped idx: idx[p, b, j*8+q] = code[q*16 + p%16, 8b+j] + 1024*j
            ix = ixpool.tile([P, NCALL, NUM_TABLE], dt.int16, tag="idx")
            idx16[k] = ix
            ix_r = ix[:].rearrange("p b (j q) -> p b j q", q=8)
            off_r = offp_sb[:].rearrange("p (b j) -> p b j", j=NBLK)
            for q in range(8):
                shuf = zpsum.tile([P, NUM_TABLE], dt.float32, tag="shuf",
                                  bufs=1)
                nc.tensor.matmul(shuf[:], selr_sb[:, q, :], code_k[:],
                                 start=True, stop=True)
                nc.vector.scalar_tensor_tensor(
                    ix_r[:, :, :, q],
                    shuf[:].rearrange("p (b j) -> p b j", j=NBLK),
                    1.0, off_r, op0=Alu.mult, op1=Alu.add)

        def emit_gathers(k):
            gts = []
            for b in range(NCALL):
                g_t = gpool.tile([P, NBLK, OUT], dt.int8, tag="g")
                nc.gpsimd.dma_gather(
                    out_ap=g_t[:],
                    in_ap=tabs[b * NBLK * TABLE_SIZE:
                               (b + 1) * NBLK * TABLE_SIZE, :],
                    idxs_ap=idx16[k][:, b, :],
                    num_idxs=NBLK * P,
                    num_idxs_reg=NBLK * P,
                    elem_size=OUT,
                    queue_num=((k * NCALL + b) % 2) if two_queues else 0,
                    single_packet=not two_queues,
                )
                gts.append(g_t)
            return gts

        def consume_block(k, b, g_t, acc, sacc):
            g8 = g_t[:].bitcast(dt.float8e3)
            for j in range(NBLK):
                t = b * NBLK + j
                path = PATHS[t]
                if path == "A":
                    dg = dgpool.tile([P, P], dt.float16, tag="dg")
                    nc.scalar.activation(dg[:], id16_sb[:], Act.Copy,
                                         scale=score_sb[:, t, k:k + 1])
                    for (n0, n1) in ((0, 512), (512, OUT)):
                        nc.tensor.matmul(acc[:, n0:n1], dg[:], g8[:, j, n0:n1],
                                         start=(t == PE_FIRST),
                                         stop=(t == PE_LAST))
                elif path == "B":
                    gs = gspool.tile([P, OUT], dt.float16, tag="gs")
                    nc.scalar.activation(gs[:], g8[:, j, :], Act.Copy,
                                         scale=score_sb[:, t, k:k + 1])
                    for (n0, n1) in ((0, 512), (512, OUT)):
                        nc.tensor.matmul(acc[:, n0:n1], id16_sb[:],
                                         gs[:, n0:n1],
                                         start=(t == PE_FIRST),
                                         stop=(t == PE_LAST))
                else:
                    if t == C_FIRST:
                        nc.vector.tensor_scalar(
                            sacc[:], g8[:, j, :], scoreq_sb[:, t, k:k + 1],
                            None, op0=Alu.mult)
                    else:
                        nc.vector.scalar_tensor_tensor(
                            sacc[:], g8[:, j, :], scoreq_sb[:, t, k:k + 1],
                            sacc[:], op0=Alu.mult, op1=Alu.add)

        # ---- software-pipelined main loop (2 groups deep) ----
        phase1_z(0)
        phase1_post(0)
        phase1_z(1)
        phase1_post(1)
        for k in range(NT):
            gts = emit_gathers(k)
            acc = accp.tile([P, OUT], dt.float32, tag="acc")
            sacc = sapool.tile([P, OUT], dt.float32, tag="sacc")
            for b in range(2):
                consume_block(k, b, gts[b], acc, sacc)
            if k + 2 < NT:
                phase1_z(k + 2)
            for b in range(2, 6):
                consume_block(k, b, gts[b], acc, sacc)
            if k + 2 < NT:
                phase1_post(k + 2)
            for b in range(6, NCALL):
                consume_block(k, b, gts[b], acc, sacc)
            # merge: o = psum * 2^-7 (ACT), o += sacc (GPSIMD), dma out
            o_t = opool.tile([P, OUT], dt.float32, tag="o")
            nc.scalar.activation(o_t[:], acc[:], Act.Copy, scale=DEQ)
            nc.gpsimd.tensor_tensor(o_t[:], o_t[:], sacc[:], op=Alu.add)
            nc.sync.dma_start(out_ap[:, k, :], o_t[:])


def host_inputs(hidden_states, W_proj, b_proj, tables):
    """Build the 8 per-core input maps from full problem inputs."""
    bf16 = ml_dtypes.bfloat16
    x = np.asarray(hidden_states, dtype=np.float32).reshape(N_TOKENS, HIDDEN)
    xh = x.astype(bf16)
    xl = (x - xh.astype(np.float32)).astype(bf16)
    W = np.asarray(W_proj, dtype=np.float32)
    Wh = np.ascontiguousarray(W.astype(bf16))
    Wl = np.ascontiguousarray((W - Wh.astype(np.float32)).astype(bf16))
    b = np.asarray(b_proj, dtype=np.float32)[None, :]
    bph = np.ascontiguousarray(b.astype(bf16))
    bpl = np.ascontiguousarray((b - bph.astype(np.float32)).astype(bf16))
    tabs_f = np.asarray(tables, dtype=np.float32)
    tabs8 = np.ascontiguousarray(
        (tabs_f * FP8_SCALE).astype(ml_dtypes.float8_e3m4).view(np.int8))
    pow2 = (2.0 ** np.arange(CODE_LEN, dtype=np.float32))
    pmat = np.tile(np.tile(pow2, NUM_TABLE)[None, :], (P, 1)).astype(np.float32)
    pmat = np.ascontiguousarray(pmat)
    id16 = np.eye(P, dtype=np.float16)
    selr = np.zeros((P, NT, P), dtype=np.float16)
    for q in range(8):
        for m in range(P):
            selr[q * 16 + (m % 16), q, m] = 1.0
    offp = np.tile(
        (1024.0 * (np.arange(NUM_TABLE) % NBLK)).astype(np.float32)[None, :],
        (P, 1))
    offp = np.ascontiguousarray(offp)
    in_maps = []
    for c in range(N_CORES):
        sl = slice(c * TOK, (c + 1) * TOK)
        in_maps.append({
            "xh": np.ascontiguousarray(xh[sl].T),
            "xl": np.ascontiguousarray(xl[sl].T),
            "Wh": Wh, "Wl": Wl, "bph": bph, "bpl": bpl,
            "tabs": tabs8, "pmat": pmat, "id16": id16, "selr": selr,
            "offpat": offp,
        })
    return in_maps


def build_nc(reps=1, two_queues=True):
    nc = bacc.Bacc("TRN2", target_bir_lowering=False, debug=False,
                   num_swdge_queues=2 if two_queues else 1)
    ins = {
        "xh": nc.dram_tensor("xh", [HIDDEN, TOK], dt.bfloat16,
                             kind="ExternalInput").ap(),
        "xl": nc.dram_tensor("xl", [HIDDEN, TOK], dt.bfloat16,
                             kind="ExternalInput").ap(),
        "Wh": nc.dram_tensor("Wh", [HIDDEN, TOTAL_DIM], dt.bfloat16,
                             kind="ExternalInput").ap(),
        "Wl": nc.dram_tensor("Wl", [HIDDEN, TOTAL_DIM], dt.bfloat16,
                             kind="ExternalInput").ap(),
        "bph": nc.dram_tensor("bph", [1, TOTAL_DIM], dt.bfloat16,
                              kind="ExternalInput").ap(),
        "bpl": nc.dram_tensor("bpl", [1, TOTAL_DIM], dt.bfloat16,
                              kind="ExternalInput").ap(),
        "tabs": nc.dram_tensor("tabs", [NUM_TABLE * TABLE_SIZE, OUT],
                               dt.int8, kind="ExternalInput").ap(),
        "pmat": nc.dram_tensor("pmat", [P, TOTAL_DIM], dt.float32,
                               kind="ExternalInput").ap(),
        "id16": nc.dram_tensor("id16", [P, P], dt.float16,
                               kind="ExternalInput").ap(),
        "selr": nc.dram_tensor("selr", [P, NT, P], dt.float16,
                               kind="ExternalInput").ap(),
        "offpat": nc.dram_tensor("offpat", [P, NUM_TABLE], dt.float32,
                                 kind="ExternalInput").ap(),
    }
    out_ap = nc.dram_tensor("out", [P, NT, OUT], dt.float32,
                            kind="ExternalOutput").ap()
    with tile.TileContext(nc) as tc:
        for _ in range(reps):
            emit_device_kernel(tc, out_ap, ins, two_queues=two_queues)
    nc.compile()
    return nc


_NC_CACHE = {}


def kernel(hidden_states, W_proj, b_proj, tables, bias, _trace=False):
    if "nc" not in _NC_CACHE:
        _NC_CACHE["nc"] = build_nc()
    nc = _NC_CACHE["nc"]
    in_maps = host_inputs(hidden_states, W_proj, b_proj, tables)
    res = run_bass_kernel_spmd(nc, in_maps, core_ids=list(range(N_CORES)),
                               trace=_trace)
    _NC_CACHE["last_results"] = res
    bias_f = np.asarray(bias, dtype=np.float32)
    parts = []
    for c in range(N_CORES):
        o = res.results[c]["out"]  # [128, 8, 1024], token d = k*128+part
        parts.append(np.transpose(o, (1, 0, 2)).reshape(TOK, OUT))
    full = np.concatenate(parts, axis=0) + bias_f[None, :]
    return full.reshape(B, S, OUT).astype(np.float32)


# revision 6
# speedup vs baseline: 3.1682x; 3.1682x over previous
"""Trainium2 Bass kernel for nn_MemoryLayer (scatter_memory).

Reference computation (per token, N = B*S = 8192 tokens):
  z = x @ W_proj + b_proj                  # [N, 640]
  factor = sigmoid(2*|z|)  (== (1+tanh|z|)/2), per element
  score[n, t] = prod_l factor[n, t*10+l]   # [N, 64]
  code[n, t]  = sum_l (z[n, t*10+l] > 0) * 2^l   # bucket in [0, 1024)
  out[n] = sum_t tables[t*1024 + code[n,t]] * score[n,t] + bias

Sharding: data-parallel over tokens (1024 tokens per core, 8 cores).

Design (v4):
- Tables stored in HBM as fp8 E3M4 (x128 scale): the gather stream stays
  at 1 byte/elem AND gathered rows feed PE / ACT / DVE directly, with no
  mandatory int8->fp16 upconversion pass.
- z projection runs as a 3-pass bf16 split (xh@Wh + xh@Wl + xl@Wh),
  4/3x faster than the PE's 4-pass true-fp32 mode at near-fp32 accuracy
  (sign(z) flips vs the f32 reference add <4e-4 to the output rel err).
- One flat software pipeline over reps*8 token-tile groups (128 tokens
  each), 2 groups deep: while group g's tables accumulate, group g+2's
  projection runs, so the gather DMA stream never starves.  All tile
  pools are created once and ring across reps (no per-rep pool barriers).
- Gathers batch 8 tables per SWDGE call (1024 rows of 1KB): descgen is
  994ns fixed + 0.34ns/descriptor, so big calls amortize the fixed cost.
  dynamic_dma_scratch_size is raised 4x so a 1024-descriptor call does
  not fill the Q7 descriptor ring.
- The weighted sum over the 64 tables is split across three statically
  balanced engine paths (per table t, per group):
    A: ACT builds diag(score) fp16; PE: psum += diag.T @ g_fp8
    B: ACT: gs = fp16(g * score) fused;  PE: psum += I.T @ gs
    C: DVE: sacc += g * (score * 2^-7)   (scalar_tensor_tensor, f32)
  Merge per group: ACT: o = psum * 2^-7; DVE: o += sacc; the out store
  is issued from the ACT HWDGE queue so the SP queue stays dedicated to
  input prefetch.
- score = prod_l sigmoid(2|z|) via DVE multiply-reduce (no Ln/Exp), so
  ACT only uses {Abs, Sigmoid, Copy} -- one activation table set, no
  LoadActFuncSet churn.

dma_gather consumes int16 indices in a 16-partition wrapped layout:
unwrapped[i] = idxs[i % 16, i // 16], row i lands at out[i % 128,
i // 128, :].  With position i = j*128 + p (j = table-in-block,
p = partition/token), the required index column is c = j*8 + p//16 with
value code[p, t=8b+j] + 1024*j.  The cross-partition shuffle
idx[p%16 (replicated), j*8 + p//16] = code[p] is done on the tensor
engine with 8 selector matmuls (SELR_q[p, m] = [p == q*16 + m%16]), and
a strided DVE scalar_tensor_tensor writes the int16 indices while adding
the 1024*j block offset.

PSUM budget (8 banks): zA ring 2 x [128,320] (2) + zB 1 (1) + shuf 1 (1)
+ acc ring 2 x [128,1024] (4).
"""

import numpy as np
import ml_dtypes

import concourse.bacc as bacc
import concourse.bass as bass
import concourse.mybir as mybir
import concourse.tile as tile
from concourse.bass_utils import run_bass_kernel_spmd

# Problem constants (hardcoded per contest rules).
B, S = 4, 2048
HIDDEN = 1024
OUT = 1024
NUM_TABLE = 64
CODE_LEN = 10
TABLE_SIZE = 1024
TOTAL_DIM = NUM_TABLE * CODE_LEN  # 640
HALF_DIM = TOTAL_DIM // 2         # 320 = 32 tables

N_CORES = 8
N_TOKENS = B * S              # 8192
TOK = N_TOKENS // N_CORES     # 1024 tokens per core
P = 128                       # partitions
NT = TOK // P                 # 8 token tiles (= pipeline groups) per core
KCH = HIDDEN // P             # 8 contraction chunks
NBLK = 8                      # tables per gather call
NCALL = NUM_TABLE // NBLK     # 8 gather calls per group

FP8_SCALE = 128.0             # tables stored as e3m4(v * 128)
DEQ = 1.0 / FP8_SCALE

# Static path assignment per table: A (PE+diag), B (ACT conv + PE id),
# C (DVE sbuf accumulate).
N_A, N_B, N_C = 22, 20, 22


def _make_paths():
    # Weighted round-robin spread of A/B/C over the 64 tables.
    counts = {"A": N_A, "B": N_B, "C": N_C}
    used = {k: 0 for k in counts}
    paths = []
    for t in range(NUM_TABLE):
        best, bestv = None, None
        for kk, c in counts.items():
            v = c * (t + 1) / NUM_TABLE - used[kk]
            if bestv is None or v > bestv:
                best, bestv = kk, v
        paths.append(best)
        used[best] += 1
    return paths


PATHS = _make_paths()
PE_TABLES = [t for t in range(NUM_TABLE) if PATHS[t] in ("A", "B")]
C_TABLES = [t for t in range(NUM_TABLE) if PATHS[t] == "C"]
PE_FIRST, PE_LAST = PE_TABLES[0], PE_TABLES[-1]
C_FIRST = C_TABLES[0] if C_TABLES else -1

dt = mybir.dt
Alu = mybir.AluOpType
Act = mybir.ActivationFunctionType
Axis = mybir.AxisListType


def emit_kernel(tc, out_ap, ins, reps, two_queues=True):
    """Emit the full multi-rep pipeline. ins: dict name -> bass.AP."""
    nc = tc.nc
    from contextlib import ExitStack

    with ExitStack() as ctx:
        const = ctx.enter_context(tc.tile_pool(name="const", bufs=1))

        # --- static constants, loaded once ---
        Wh_sb = const.tile([P, KCH, TOTAL_DIM], dt.bfloat16)
        nc.sync.dma_start(Wh_sb[:],
                          ins["Wh"][:].rearrange("(c p) h -> p c h", c=KCH))
        Wl_sb = const.tile([P, KCH, TOTAL_DIM], dt.bfloat16)
        nc.sync.dma_start(Wl_sb[:],
                          ins["Wl"][:].rearrange("(c p) h -> p c h", c=KCH))
        Pm_sb = const.tile([P, TOTAL_DIM], dt.float32)
        nc.sync.dma_start(Pm_sb[:], ins["pmat"][:])
        id16_sb = const.tile([P, P], dt.float16)
        nc.sync.dma_start(id16_sb[:], ins["id16"][:])
        selr_sb = const.tile([P, NT, P], dt.float16)
        nc.sync.dma_start(selr_sb[:], ins["selr"][:])
        offp_sb = const.tile([P, NUM_TABLE], dt.float32)
        nc.sync.dma_start(offp_sb[:], ins["offpat"][:])
        bph_sb = const.tile([1, TOTAL_DIM], dt.bfloat16)
        nc.sync.dma_start(bph_sb[:], ins["bph"][:])
        bpl_sb = const.tile([1, TOTAL_DIM], dt.bfloat16)
        nc.sync.dma_start(bpl_sb[:], ins["bpl"][:])
        ones_sb = const.tile([1, P], dt.bfloat16)
        nc.vector.memset(ones_sb[:], 1.0)
        tabs = ins["tabs"]

        # --- ring pools, shared across reps ---
        xpool = ctx.enter_context(tc.tile_pool(name="xin", bufs=1))
        scpool = ctx.enter_context(tc.tile_pool(name="scor", bufs=2))
        gpool = ctx.enter_context(tc.tile_pool(name="gbuf", bufs=5))
        gspool = ctx.enter_context(tc.tile_pool(name="gsc", bufs=4))
        dgpool = ctx.enter_context(tc.tile_pool(name="diag", bufs=8))
        sapool = ctx.enter_context(tc.tile_pool(name="sacc", bufs=3))
        opool = ctx.enter_context(tc.tile_pool(name="outs", bufs=3))
        cdpool = ctx.enter_context(tc.tile_pool(name="code", bufs=3))
        ixpool = ctx.enter_context(tc.tile_pool(name="idx", bufs=3))
        ztmp = ctx.enter_context(tc.tile_pool(name="ztmp", bufs=2))
        # PSUM: zA(2) + zB(1) + shuf(1) + acc(2x2) = 8 banks
        zpsum = ctx.enter_context(tc.tile_pool(name="zp", bufs=1,
                                               space="PSUM"))
        accp = ctx.enter_context(tc.tile_pool(name="acc", bufs=2,
                                              space="PSUM"))

        rep_state = {}   # r -> (xh_sb, xl_sb, score_sb, scoreq_sb)
        zAs, zBs, codes, idx16 = {}, {}, {}, {}

        def ensure_rep(r):
            if r in rep_state or r >= reps:
                return
            xh_sb = xpool.tile([P, KCH, TOK], dt.bfloat16, tag="xh",
                               name=f"xh_{r}")
            xl_sb = xpool.tile([P, KCH, TOK], dt.bfloat16, tag="xl",
                               name=f"xl_{r}")
            for k in range(NT):
                sl = slice(k * P, (k + 1) * P)
                nc.sync.dma_start(
                    xh_sb[:, :, sl],
                    ins["xh"][:, sl].rearrange("(c p) h -> p c h", c=KCH))
                nc.sync.dma_start(
                    xl_sb[:, :, sl],
                    ins["xl"][:, sl].rearrange("(c p) h -> p c h", c=KCH))
            score_sb = scpool.tile([P, NUM_TABLE, NT], dt.float32,
                                   tag="score", name=f"score_{r}")
            scoreq_sb = scpool.tile([P, NUM_TABLE, NT], dt.float32,
                                    tag="scoreq", name=f"scoreq_{r}")
            rep_state[r] = (xh_sb, xl_sb, score_sb, scoreq_sb)

        def phase1_z(g):
            r, k = divmod(g, NT)
            ensure_rep(r)
            xh_sb, xl_sb, _, _ = rep_state[r]
            za = zpsum.tile([P, HALF_DIM], dt.float32, tag="zA", bufs=2,
                            name=f"zA_{g}")
            zb = zpsum.tile([P, HALF_DIM], dt.float32, tag="zB", bufs=1,
                            name=f"zB_{g}")
            zAs[g], zBs[g] = za, zb
            sl = slice(k * P, (k + 1) * P)
            for c in range(KCH):
                first = c == 0
                nc.tensor.matmul(za[:], xh_sb[:, c, sl],
                                 Wh_sb[:, c, :HALF_DIM],
                                 start=first, stop=False)
                nc.tensor.matmul(za[:], xh_sb[:, c, sl],
                                 Wl_sb[:, c, :HALF_DIM],
                                 start=False, stop=False)
                nc.tensor.matmul(zb[:], xh_sb[:, c, sl],
                                 Wh_sb[:, c, HALF_DIM:],
                                 start=first, stop=False)
                nc.tensor.matmul(zb[:], xh_sb[:, c, sl],
                                 Wl_sb[:, c, HALF_DIM:],
                                 start=False, stop=False)
                nc.tensor.matmul(za[:], xl_sb[:, c, sl],
                                 Wh_sb[:, c, :HALF_DIM],
                                 start=False, stop=False)
                nc.tensor.matmul(zb[:], xl_sb[:, c, sl],
                                 Wh_sb[:, c, HALF_DIM:],
                                 start=False, stop=False)
            nc.tensor.matmul(za[:], ones_sb[0:1, :], bph_sb[0:1, :HALF_DIM],
                             start=False, stop=False)
            nc.tensor.matmul(zb[:], ones_sb[0:1, :], bph_sb[0:1, HALF_DIM:],
                             start=False, stop=False)
            nc.tensor.matmul(za[:], ones_sb[0:1, :], bpl_sb[0:1, :HALF_DIM],
                             start=False, stop=True)
            nc.tensor.matmul(zb[:], ones_sb[0:1, :], bpl_sb[0:1, HALF_DIM:],
                             start=False, stop=True)

        def phase1_post(g):
            r, k = divmod(g, NT)
            _, _, score_sb, scoreq_sb = rep_state[r]
            code_k = cdpool.tile([P, NUM_TABLE], dt.float16, tag="code",
                                 name=f"code_{g}")
            codes[g] = code_k
            for h, z in ((0, zAs[g]), (1, zBs[g])):
                tsl = slice(32 * h, 32 * (h + 1))
                dsl = slice(HALF_DIM * h, HALF_DIM * (h + 1))
                ab = ztmp.tile([P, HALF_DIM], dt.float32, tag="ab",
                               name=f"ab_{g}_{h}")
                nc.scalar.activation(ab[:], z[:], Act.Abs)
                fa = ztmp.tile([P, HALF_DIM], dt.float32, tag="fa",
                               name=f"fa_{g}_{h}")
                nc.scalar.activation(fa[:], ab[:], Act.Sigmoid, scale=2.0)
                nc.vector.tensor_reduce(
                    score_sb[:, tsl, k],
                    fa[:].rearrange("p (t l) -> p t l", l=CODE_LEN),
                    axis=Axis.X, op=Alu.mult,
                )
                nc.vector.tensor_scalar(
                    scoreq_sb[:, tsl, k], score_sb[:, tsl, k], DEQ, None,
                    op0=Alu.mult)
                bcs = ztmp.tile([P, HALF_DIM], dt.float32, tag="bc",
                                name=f"bc_{g}_{h}")
                nc.vector.scalar_tensor_tensor(
                    bcs[:], z[:], 0.0, Pm_sb[:, dsl],
                    op0=Alu.is_gt, op1=Alu.mult)
                with nc.allow_low_precision(
                        reason="codes are integers <= 1023, exact in fp16"):
                    nc.vector.tensor_reduce(
                        code_k[:, tsl],
                        bcs[:].rearrange("p (t l) -> p t l", l=CODE_LEN),
                        axis=Axis.X, op=Alu.add,
                    )
            # wrapped idx: idx[p, b, j*8+q] = code[q*16+p%16, 8b+j] + 1024*j
            ix = ixpool.tile([P, NCALL, NUM_TABLE], dt.int16, tag="idx",
                             name=f"idx_{g}")
            idx16[g] = ix
            ix_r = ix[:].rearrange("p b (j q) -> p b j q", q=8)
            off_r = offp_sb[:].rearrange("p (b j) -> p b j", j=NBLK)
            for q in range(8):
                shuf = zpsum.tile([P, NUM_TABLE], dt.float32, tag="shuf",
                                  bufs=1, name=f"shuf_{g}_{q}")
                nc.tensor.matmul(shuf[:], selr_sb[:, q, :], code_k[:],
                                 start=True, stop=True)
                nc.vector.scalar_tensor_tensor(
                    ix_r[:, :, :, q],
                    shuf[:].rearrange("p (b j) -> p b j", j=NBLK),
                    1.0, off_r, op0=Alu.mult, op1=Alu.add)

        def emit_gathers(g):
            gts = []
            for b in range(NCALL):
                g_t = gpool.tile([P, NBLK, OUT], dt.int8, tag="g",
                                 name=f"g_{g}_{b}")
                nc.gpsimd.dma_gather(
                    out_ap=g_t[:],
                    in_ap=tabs[b * NBLK * TABLE_SIZE:
                               (b + 1) * NBLK * TABLE_SIZE, :],
                    idxs_ap=idx16[g][:, b, :],
                    num_idxs=NBLK * P,
                    num_idxs_reg=NBLK * P,
                    elem_size=OUT,
                    queue_num=((g * NCALL + b) % 2) if two_queues else 0,
                    single_packet=not two_queues,
                )
                gts.append(g_t)
            return gts

        def consume_block(g, b, g_t, acc, sacc):
            r, k = divmod(g, NT)
            _, _, score_sb, scoreq_sb = rep_state[r]
            g8 = g_t[:].bitcast(dt.float8e3)
            for j in range(NBLK):
                t = b * NBLK + j
                path = PATHS[t]
                if path == "A":
                    dg = dgpool.tile([P, P], dt.float16, tag="dg",
                                     name=f"dg_{g}_{t}")
                    nc.scalar.activation(dg[:], id16_sb[:], Act.Copy,
                                         scale=score_sb[:, t, k:k + 1])
                    for (n0, n1) in ((0, 512), (512, OUT)):
                        nc.tensor.matmul(acc[:, n0:n1], dg[:],
                                         g8[:, j, n0:n1],
                                         start=(t == PE_FIRST),
                                         stop=(t == PE_LAST))
                elif path == "B":
                    gs = gspool.tile([P, OUT], dt.float16, tag="gs",
                                     name=f"gs_{g}_{t}")
                    nc.scalar.activation(gs[:], g8[:, j, :], Act.Copy,
                                         scale=score_sb[:, t, k:k + 1])
                    for (n0, n1) in ((0, 512), (512, OUT)):
                        nc.tensor.matmul(acc[:, n0:n1], id16_sb[:],
                                         gs[:, n0:n1],
                                         start=(t == PE_FIRST),
                                         stop=(t == PE_LAST))
                else:
                    if t == C_FIRST:
                        nc.vector.tensor_scalar(
                            sacc[:], g8[:, j, :], scoreq_sb[:, t, k:k + 1],
                            None, op0=Alu.mult)
                    else:
                        nc.vector.scalar_tensor_tensor(
                            sacc[:], g8[:, j, :], scoreq_sb[:, t, k:k + 1],
                            sacc[:], op0=Alu.mult, op1=Alu.add)

        # ---- flat software-pipelined loop over reps*NT groups ----
        total = reps * NT
        phase1_z(0)
        phase1_post(0)
        phase1_z(1)
        phase1_post(1)
        for g in range(total):
            r, k = divmod(g, NT)
            gts = emit_gathers(g)
            acc = accp.tile([P, OUT], dt.float32, tag="acc", name=f"acc_{g}")
            sacc = sapool.tile([P, OUT], dt.float32, tag="sacc",
                               name=f"sacc_{g}")
            for b in range(2):
                consume_block(g, b, gts[b], acc, sacc)
            if g + 2 < total:
                phase1_z(g + 2)
            for b in range(2, 6):
                consume_block(g, b, gts[b], acc, sacc)
            if g + 2 < total:
                phase1_post(g + 2)
            for b in range(6, NCALL):
                consume_block(g, b, gts[b], acc, sacc)
            # merge: o = psum * 2^-7 (ACT); o += sacc (DVE); store via
            # the ACT HWDGE queue (SP stays dedicated to input prefetch)
            o_t = opool.tile([P, OUT], dt.float32, tag="o", name=f"o_{g}")
            nc.scalar.activation(o_t[:], acc[:], Act.Copy, scale=DEQ)
            nc.vector.tensor_tensor(o_t[:], o_t[:], sacc[:], op=Alu.add)
            nc.scalar.dma_start(out_ap[:, k, :], o_t[:])


def host_inputs(hidden_states, W_proj, b_proj, tables):
    """Build the 8 per-core input maps from full problem inputs."""
    bf16 = ml_dtypes.bfloat16
    x = np.asarray(hidden_states, dtype=np.float32).reshape(N_TOKENS, HIDDEN)
    xh = x.astype(bf16)
    xl = (x - xh.astype(np.float32)).astype(bf16)
    W = np.asarray(W_proj, dtype=np.float32)
    Wh = np.ascontiguousarray(W.astype(bf16))
    Wl = np.ascontiguousarray((W - Wh.astype(np.float32)).astype(bf16))
    b = np.asarray(b_proj, dtype=np.float32)[None, :]
    bph = np.ascontiguousarray(b.astype(bf16))
    bpl = np.ascontiguousarray((b - bph.astype(np.float32)).astype(bf16))
    tabs_f = np.asarray(tables, dtype=np.float32)
    tabs8 = np.ascontiguousarray(
        (tabs_f * FP8_SCALE).astype(ml_dtypes.float8_e3m4).view(np.int8))
    pow2 = (2.0 ** np.arange(CODE_LEN, dtype=np.float32))
    pmat = np.tile(np.tile(pow2, NUM_TABLE)[None, :], (P, 1)).astype(np.float32)
    pmat = np.ascontiguousarray(pmat)
    id16 = np.eye(P, dtype=np.float16)
    selr = np.zeros((P, NT, P), dtype=np.float16)
    for q in range(8):
        for m in range(P):
            selr[q * 16 + (m % 16), q, m] = 1.0
    offp = np.tile(
        (1024.0 * (np.arange(NUM_TABLE) % NBLK)).astype(np.float32)[None, :],
        (P, 1))
    offp = np.ascontiguousarray(offp)
    in_maps = []
    for c in range(N_CORES):
        sl = slice(c * TOK, (c + 1) * TOK)
        in_maps.append({
            "xh": np.ascontiguousarray(xh[sl].T),
            "xl": np.ascontiguousarray(xl[sl].T),
            "Wh": Wh, "Wl": Wl, "bph": bph, "bpl": bpl,
            "tabs": tabs8, "pmat": pmat, "id16": id16, "selr": selr,
            "offpat": offp,
        })
    return in_maps


def build_nc(reps=1, two_queues=True):
    nc = bacc.Bacc("TRN2", target_bir_lowering=False, debug=False,
                   num_swdge_queues=2 if two_queues else 1,
                   dynamic_dma_scratch_size=65536)
    ins = {
        "xh": nc.dram_tensor("xh", [HIDDEN, TOK], dt.bfloat16,
                             kind="ExternalInput").ap(),
        "xl": nc.dram_tensor("xl", [HIDDEN, TOK], dt.bfloat16,
                             kind="ExternalInput").ap(),
        "Wh": nc.dram_tensor("Wh", [HIDDEN, TOTAL_DIM], dt.bfloat16,
                             kind="ExternalInput").ap(),
        "Wl": nc.dram_tensor("Wl", [HIDDEN, TOTAL_DIM], dt.bfloat16,
                             kind="ExternalInput").ap(),
        "bph": nc.dram_tensor("bph", [1, TOTAL_DIM], dt.bfloat16,
                              kind="ExternalInput").ap(),
        "bpl": nc.dram_tensor("bpl", [1, TOTAL_DIM], dt.bfloat16,
                              kind="ExternalInput").ap(),
        "tabs": nc.dram_tensor("tabs", [NUM_TABLE * TABLE_SIZE, OUT],
                               dt.int8, kind="ExternalInput").ap(),
        "pmat": nc.dram_tensor("pmat", [P, TOTAL_DIM], dt.float32,
                               kind="ExternalInput").ap(),
        "id16": nc.dram_tensor("id16", [P, P], dt.float16,
                               kind="ExternalInput").ap(),
        "selr": nc.dram_tensor("selr", [P, NT, P], dt.float16,
                               kind="ExternalInput").ap(),
        "offpat": nc.dram_tensor("offpat", [P, NUM_TABLE], dt.float32,
                                 kind="ExternalInput").ap(),
    }
    out_ap = nc.dram_tensor("out", [P, NT, OUT], dt.float32,
                            kind="ExternalOutput").ap()
    with tile.TileContext(nc) as tc:
        emit_kernel(tc, out_ap, ins, reps, two_queues=two_queues)
    nc.compile()
    return nc


_NC_CACHE = {}


def kernel(hidden_states, W_proj, b_proj, tables, bias, _trace=False):
    if "nc" not in _NC_CACHE:
        _NC_CACHE["nc"] = build_nc()
    nc = _NC_CACHE["nc"]
    in_maps = host_inputs(hidden_states, W_proj, b_proj, tables)
    res = run_bass_kernel_spmd(nc, in_maps, core_ids=list(range(N_CORES)),
                               trace=_trace)
    _NC_CACHE["last_results"] = res
    bias_f = np.asarray(bias, dtype=np.float32)
    parts = []
    for c in range(N_CORES):
        o = res.results[c]["out"]  # [128, 8, 1024], token d = k*128+part
        parts.append(np.transpose(o, (1, 0, 2)).reshape(TOK, OUT))
    full = np.concatenate(parts, axis=0) + bias_f[None, :]
    return full.reshape(B, S, OUT).astype(np.float32)
